# revision 13
# baseline (speedup 1.0000x reference)
"""Trainium2 Bass kernel for prefix-KV causal attention (nn_Attn_38757784879167).

Sharding: 8 cores <- (b, h) pairs (B=2 x H=4). Each core runs the full
attention for one (batch, head): QKV projection, S^T-layout flash attention
(scores computed transposed: keys on partitions, queries on free dim -> no
P transposes needed), PV + rowsum fused via an appended ones-row on V,
normalization + per-head out-projection partial. Host sums the 4 per-head
partials per batch (the out-projection "all-reduce" gather step).

Layout core ideas:
  - All per-core inputs are host-packed into ONE [128, NW] f32 tensor ->
    a single input DMA (fp32 matmuls can carry only one sync wait, so
    downstream waits must collapse onto one DMA lane).
  - S^T tile [128 keys, 512 queries] = matmul(lhsT=kT[:,chunk] [32,128],
    rhs=qT[:,qtile] [32,512]) in fp32r (1 cyc/row at N>=256).
  - exp on ScalarE in 3-chunk batches ([128,1536] PSUM->SBUF) to amortize
    per-instruction overhead; softmax max-subtraction is skipped (scores
    *1/sqrt(hd) are ~N(0,1), global max ~9.5 -> exp is fp32-safe).
  - v' = [v | 1] ([128,33] chunks) makes one PV matmul produce both
    ctx^T (rows 0..31) and the softmax row-sums (row 32).
  - shifted-causal mask applied as a 0/1 multiply on the 4 diagonal chunks
    of each query tile (mask tiles precomputed on host).
"""

import math
import os

import numpy as np

B = 2
T = 4096
D = 128
H = 4
HD = 32
PRE = 2048
CH = 128  # keys per chunk (partition dim of S^T tiles)
QT = 512  # queries per tile (free dim of S^T tiles)
GRP = 3  # chunks per exp batch (3 PSUM banks)

_CACHE = {}


def _offsets(T, PRE):
    """Column offsets into the packed [128, NW] input tensor."""
    diag = QT // CH
    nch = (T + PRE) // CH
    o = {}
    o["x"] = 0  # xT [128, T]
    o["pk"] = T  # pkT rows 0..31 [*, PRE]
    o["pv"] = T + PRE  # full vS image [128, nch*(HD+1)]: pv chunks + ones cols
    o["m"] = o["pv"] + nch * (HD + 1)  # mask [128, diag*QT]
    o["wq"] = o["m"] + diag * QT
    o["wk"] = o["wq"] + HD
    o["wv"] = o["wk"] + HD
    o["wo"] = o["wv"] + HD  # wout rows 0..31 [*, D]
    o["nw"] = o["wo"] + D
    return o


def build_attn(T=T, PRE=PRE, grp=GRP):
    """Build + compile the per-core Bacc module. Returns nc."""
    from contextlib import ExitStack

    import concourse.mybir as mybir
    import concourse.tile as tile
    from concourse import bacc

    f32 = mybir.dt.float32
    f32r = mybir.dt.float32r
    EXP = mybir.ActivationFunctionType.Exp
    TK = PRE + T
    NCH = TK // CH
    NQT = T // QT
    DIAG = QT // CH
    SCALE = 1.0 / math.sqrt(HD)
    O = _offsets(T, PRE)

    nc = bacc.Bacc("TRN2", target_bir_lowering=False, debug=False)

    pack_d = nc.dram_tensor("pack", [128, O["nw"]], f32, kind="ExternalInput")
    out_d = nc.dram_tensor("out", [T, D], f32, kind="ExternalOutput")

    with tile.TileContext(nc) as tc, ExitStack() as ctx:
        const = ctx.enter_context(tc.tile_pool(name="const", bufs=1))
        packed = const.tile([128, O["nw"]], f32, tag="pack")
        qT_s = const.tile([HD, T], f32r, tag="qT")
        kT_s = const.tile([HD, TK], f32r, tag="kT")
        vS_s = const.tile([128, NCH * (HD + 1)], f32r, tag="vS")
        mask_s = const.tile([CH, DIAG * QT], f32r, tag="mask")
        ones_s = const.tile([1, 1], f32, tag="ones")

        nc.sync.dma_start(packed[:], pack_d[:])

        xT = packed[:, O["x"] : O["x"] + T]
        wq = packed[:, O["wq"] : O["wq"] + HD]
        wk = packed[:, O["wk"] : O["wk"] + HD]
        wv = packed[:, O["wv"] : O["wv"] + HD]
        wout = packed[0:HD, O["wo"] : O["wo"] + D]

        vS_3d = vS_s[:].rearrange("p (c e) -> p c e", e=HD + 1)
        nc.vector.tensor_copy(kT_s[:, 0:PRE], packed[0:HD, O["pk"] : O["pk"] + PRE])
        # vS image: pv chunks + ALL ones-columns pre-packed on host
        nc.vector.tensor_copy(vS_s[:], packed[:, O["pv"] : O["pv"] + NCH * (HD + 1)])
        nc.vector.tensor_copy(mask_s[:], packed[:, O["m"] : O["m"] + DIAG * QT])
        nc.vector.memset(ones_s[:], 1.0)

        # ---- preamble: q/k/v projections (full fp32 for precision) ----
        with tc.tile_pool(name="prePS", bufs=4, space="PSUM") as pre_ps:
            for t0 in range(0, T, QT):
                psq = pre_ps.tile([HD, QT], f32, tag="p")
                nc.tensor.matmul(psq[:], wq, xT[:, t0 : t0 + QT])
                nc.vector.tensor_copy(qT_s[:, t0 : t0 + QT], psq[:])
                psk = pre_ps.tile([HD, QT], f32, tag="p")
                nc.tensor.matmul(psk[:], wk, xT[:, t0 : t0 + QT])
                nc.scalar.copy(kT_s[:, PRE + t0 : PRE + t0 + QT], psk[:])
            for i in range(T // CH):
                psv = pre_ps.tile([CH, HD], f32, tag="p")
                nc.tensor.matmul(psv[:], xT[:, CH * i : CH * (i + 1)], wv)
                nc.vector.tensor_copy(vS_3d[:, PRE // CH + i, 0:HD], psv[:])

        # ---- attention ----
        psS = ctx.enter_context(tc.tile_pool(name="psS", bufs=2, space="PSUM"))
        psCE = ctx.enter_context(tc.tile_pool(name="psCE", bufs=1, space="PSUM"))
        psOP = ctx.enter_context(tc.tile_pool(name="psOP", bufs=1, space="PSUM"))
        ptp = ctx.enter_context(tc.tile_pool(name="pt", bufs=3))
        epp = ctx.enter_context(tc.tile_pool(name="ep", bufs=2))
        outp = ctx.enter_context(tc.tile_pool(name="outp", bufs=3))

        kT_r = kT_s[:]
        qT_r = qT_s[:]
        vS_r = vS_s[:].rearrange("p (c e) -> p c e", e=HD + 1)

        for j in range(NQT):
            nch = (PRE + QT * (j + 1)) // CH
            psC = psCE.tile([HD + 1, QT], f32, tag="ce")
            for c0 in range(0, nch, grp):
                c1 = min(c0 + grp, nch)
                w = (c1 - c0) * QT
                ps = psS.tile([CH, grp * QT], f32, tag="s")
                for c in range(c0, c1):
                    off = (c - c0) * QT
                    nc.tensor.matmul(
                        ps[:, off : off + QT],
                        kT_r[:, CH * c : CH * (c + 1)],
                        qT_r[:, QT * j : QT * (j + 1)],
                    )
                pt = ptp.tile([CH, grp * QT], f32r, tag="pt")
                nc.scalar.activation(pt[:, 0:w], ps[:, 0:w], EXP, scale=SCALE)
                for c in range(c0, c1):
                    off = (c - c0) * QT
                    d = c - (nch - DIAG)
                    if d >= 0:
                        nc.vector.tensor_mul(
                            pt[:, off : off + QT],
                            pt[:, off : off + QT],
                            mask_s[:, QT * d : QT * (d + 1)],
                        )
                    nc.tensor.matmul(
                        psC[:],
                        vS_r[:, c, :],
                        pt[:, off : off + QT],
                        start=(c == 0),
                        stop=(c == nch - 1),
                        skip_group_check=True,
                    )

            # ---- epilogue for this query tile ----
            ctxT_s = epp.tile([HD, QT], f32, tag="ctxT")
            nc.vector.tensor_copy(ctxT_s[:], psC[0:HD, :])
            rs_s = epp.tile([1, QT], f32, tag="rs")
            nc.vector.reciprocal(rs_s[:], psC[HD : HD + 1, :])
            psR = psCE.tile([128, QT // 128], f32, tag="ce")
            for jj in range(QT // 128):
                nc.tensor.matmul(
                    psR[:, jj : jj + 1],
                    rs_s[0:1, 128 * jj : 128 * (jj + 1)],
                    ones_s[:],
                )
            rec_s = epp.tile([128, QT // 128], f32, tag="rec")
            nc.vector.tensor_copy(rec_s[:], psR[:])
            psO = psOP.tile([128, (QT // 128) * D], f32, tag="o")
            for jj in range(QT // 128):
                nc.tensor.matmul(
                    psO[:, D * jj : D * (jj + 1)],
                    ctxT_s[:, 128 * jj : 128 * (jj + 1)],
                    wout,
                )
                ot = outp.tile([128, D], f32, tag="o")
                nc.vector.tensor_scalar_mul(
                    ot[:], psO[:, D * jj : D * (jj + 1)], rec_s[:, jj : jj + 1]
                )
                r0 = QT * j + 128 * jj
                nc.sync.dma_start(out_d[r0 : r0 + 128, :], ot[:])

    nc.compile()
    return nc


def _make_masks(qt=QT, ch=CH):
    diag = qt // ch
    m = np.zeros((ch, diag * qt), dtype=np.float32)
    p = np.arange(ch)[:, None]
    t = np.arange(qt)[None, :]
    for d in range(diag):
        m[:, qt * d : qt * (d + 1)] = (t >= ch * d + p).astype(np.float32)
    return m


def pack_inputs(x_b, pk_bh, pv_bh, wq, wk, wv, wout_h, Tv=T, PREv=PRE):
    """Pack one core's inputs into the [128, NW] f32 tensor."""
    O = _offsets(Tv, PREv)
    p = np.zeros((128, O["nw"]), dtype=np.float32)
    p[:, O["x"] : O["x"] + Tv] = x_b.T
    p[0:HD, O["pk"] : O["pk"] + PREv] = pk_bh.T
    nch = (Tv + PREv) // CH
    vimg = np.zeros((128, nch, HD + 1), dtype=np.float32)
    vimg[:, :, HD] = 1.0
    vimg[:, 0 : PREv // CH, 0:HD] = pv_bh.reshape(PREv // CH, CH, HD).transpose(
        1, 0, 2
    )
    p[:, O["pv"] : O["m"]] = vimg.reshape(128, -1)
    p[:, O["m"] : O["m"] + (QT // CH) * QT] = _make_masks()
    p[:, O["wq"] : O["wq"] + HD] = wq
    p[:, O["wk"] : O["wk"] + HD] = wk
    p[:, O["wv"] : O["wv"] + HD] = wv
    p[0:HD, O["wo"] : O["wo"] + D] = wout_h
    return p


def make_in_maps(x, pk, pv, Wqkv, Wout):
    in_maps = []
    for b in range(B):
        for h in range(H):
            in_maps.append(
                {
                    "pack": pack_inputs(
                        np.asarray(x[b], dtype=np.float32),
                        np.asarray(pk[b, h], dtype=np.float32),
                        np.asarray(pv[b, h], dtype=np.float32),
                        np.asarray(Wqkv[:, h * HD : (h + 1) * HD], dtype=np.float32),
                        np.asarray(
                            Wqkv[:, D + h * HD : D + (h + 1) * HD], dtype=np.float32
                        ),
                        np.asarray(
                            Wqkv[:, 2 * D + h * HD : 2 * D + (h + 1) * HD],
                            dtype=np.float32,
                        ),
                        np.asarray(Wout[h * HD : (h + 1) * HD, :], dtype=np.float32),
                    )
                }
            )
    return in_maps


def _install_ntff_shim():
    """Provide antenv.axon_hooks (absent in this image) so trace=True works.

    Replicates trn_boot._ntff_profile_via_ctypes against /opt/axon/libaxon_pjrt.so.
    """
    import contextlib
    import ctypes
    import sys
    import types

    try:
        from antenv.axon_hooks import get_axon_ntff_profile_hook  # noqa: F401

        return True
    except ImportError:
        pass
    so_path = "/opt/axon/libaxon_pjrt.so"
    if not os.path.exists(so_path):
        return False
    lib = ctypes.CDLL(so_path)
    if not hasattr(lib, "axon_start_nrt_profile"):
        return False
    lib.axon_start_nrt_profile.argtypes = [
        ctypes.POINTER(ctypes.c_int64),
        ctypes.c_size_t,
    ]
    lib.axon_start_nrt_profile.restype = ctypes.c_int64
    lib.axon_stop_nrt_profile.argtypes = [ctypes.c_char_p]
    lib.axon_stop_nrt_profile.restype = ctypes.c_int64

    @contextlib.contextmanager
    def _hook(output_dir, device_ids):
        import jax

        jax.devices()
        if device_ids:
            ids = (ctypes.c_int64 * len(device_ids))(*device_ids)
            rc = lib.axon_start_nrt_profile(ids, len(device_ids))
        else:
            rc = lib.axon_start_nrt_profile(None, 0)
        if rc != 0:
            raise RuntimeError(f"axon_start_nrt_profile rc={rc}")
        try:
            yield
        finally:
            n = lib.axon_stop_nrt_profile(str(output_dir).encode())
            if n < 0:
                raise RuntimeError(f"axon_stop_nrt_profile rc={n}")

    mod = types.ModuleType("antenv.axon_hooks")
    mod.get_axon_ntff_profile_hook = lambda: _hook
    mod.set_axon_ntff_profile_hook = lambda h: None
    sys.modules["antenv.axon_hooks"] = mod
    return True


def kernel(x, pk, pv, Wqkv, Wout):
    from concourse.bass_utils import run_bass_kernel_spmd

    if "nc" not in _CACHE:
        _CACHE["nc"] = build_attn()
    nc = _CACHE["nc"]
    in_maps = make_in_maps(x, pk, pv, Wqkv, Wout)
    trace = bool(int(os.environ.get("ATTN_TRACE", "0")))
    if trace:
        trace = _install_ntff_shim()
    res = run_bass_kernel_spmd(
        nc, in_maps, core_ids=list(range(B * H)), trace=trace
    )
    _CACHE["last_results"] = res
    out = np.zeros((B, T, D), dtype=np.float32)
    for b in range(B):
        for h in range(H):
            out[b] += res.results[b * H + h]["out"]
    return out
